# revision 4
# baseline (speedup 1.0000x reference)
"""BlockReLU Trainium2 kernel.

Full input: activation [32, 128, 112, 112] f32. Channel groups:
  [0,64): 1x1 blocks (plain ReLU), [64,96): 2x2 blocks, [96,120): 4x4 blocks,
  [120,128): identity passthrough.
A block's mask is 1 where the block's spatial sum >= 0, else 0; the mask is
broadcast over the block and multiplies the input.

Strategy: pure data parallelism over batch N across 8 NeuronCores (4 images
per core). The problem is HBM-bandwidth-bound (baseline f32-in/f32-out ran at
~374 GB/s/core, at the shared-HBM-stack roofline), so the win is TRAFFIC
REDUCTION under the 2e-2 rel-err budget:
  - All OUTPUT is written fp16 (rel err 2^-11 = 4.9e-4; host upcasts to f32).
  - G1 (1x1/ReLU, channels [0,64) = half the tensor) INPUT is fp16
    (host pre-casts): a 1x1 "mask flip" only happens when |x| is at fp16
    rounding scale, so the error stays ~5e-4*|x|.
  - G2/G3 inputs stay f32: their masks need exact block sums -- an fp16-
    perturbed sum near 0 would flip a whole block's mask and zero/unzero
    O(1)-magnitude elements (max-norm rel err ~0.5).
Traffic/core: 19.25 MB loads + 12.8 MB stores = 32.1 MB vs 51.4 MB baseline.

Per core, stream H in chunks of CHUNK_H rows. For each chunk, pack
(channel, image) pairs onto all 128 SBUF partitions so every engine op uses
every lane:
  G1 relu:  two fp16 tiles, p = c*2 + n_local (64ch x 2img) -> ScalarE Relu
            in-place, stored straight from the input tile.
  G2 2x2:   one f32 tile,  p = (c-64)*4 + n  (32ch x 4img) -> VectorE,
            masked product written to an fp16 out tile (conversion fused
            into the scalar_tensor_tensor write port).
  G3+G4:    one f32 tile,  p = (c-96)*4 + n  (32ch x 4img);
            partitions 0:96 are the 4x4-mask channels (VectorE), 96:128
            identity channels copied f32->fp16 on ScalarE.
Block sums are strided pairwise tensor_adds in f32; mask apply is a fused
scalar_tensor_tensor: out_fp16 = (sum >= 0) * x_f32.

Engine/queue layout (measured on HW in the f32 baseline; kept):
  - Loads split across both HWDGE rings (g2+g1a on nc.sync/SP, g3+g1b on
    nc.scalar/ACT). Stores kept OFF these rings: a store's sem-wait
    head-of-line-blocks later descriptor pushes, starving the load stream.
  - All stores on the SWDGE ring (nc.gpsimd.dma_start) - descriptor
    generation only.
  - Deep prefetch: input pools hold all 7 chunks (bufs=7) so loads never
    wait on store completions.
  - All DMAs are 128-partition; 96-partition DMAs cap at 12/16 SBUF ports
    and poison aggregate bandwidth (measured), hence the identity channels
    ride inside the G3 tile.
"""
import sys

if "/opt/trn_rl_repo" not in sys.path:
    sys.path.insert(0, "/opt/trn_rl_repo")

import numpy as np
from contextlib import ExitStack

import concourse.tile as tile
from concourse import bacc, mybir
from concourse.bass_utils import run_bass_kernel_spmd

N_FULL, C, H, W = 32, 128, 112, 112
N_CORES = 8
N_PER_CORE = N_FULL // N_CORES  # 4
CHUNK_H = 16

_compiled = None


def _build():
    N = N_PER_CORE
    f32 = mybir.dt.float32
    f16 = mybir.dt.float16
    nc = bacc.Bacc("TRN2", target_bir_lowering=False, debug=False)
    x1 = nc.dram_tensor("x1", [N, 64, H, W], f16, kind="ExternalInput").ap()
    x23 = nc.dram_tensor("x23", [N, 64, H, W], f32, kind="ExternalInput").ap()
    y = nc.dram_tensor("y", [N, C, H, W], f16, kind="ExternalOutput").ap()

    n_chunks = H // CHUNK_H
    F = CHUNK_H * W
    ge, mul = mybir.AluOpType.is_ge, mybir.AluOpType.mult

    with tile.TileContext(nc) as tc, ExitStack() as ctx:
        p1 = ctx.enter_context(tc.tile_pool(name="g1", bufs=7))
        p2 = ctx.enter_context(tc.tile_pool(name="g2", bufs=7))
        p3 = ctx.enter_context(tc.tile_pool(name="g3", bufs=7))
        po = ctx.enter_context(tc.tile_pool(name="out", bufs=3))
        tp = ctx.enter_context(tc.tile_pool(name="tmp", bufs=1))

        # ---- ALL loads upfront: with bufs=7 every chunk has its own
        # buffer, so the descriptor pushes have no waits and the two HWDGE
        # rings stream back-to-back. (Interleaving pushes with ACTIVATEs on
        # the ACT engine was measured to stretch that ring's load span from
        # 51us to 68us: head-of-line blocking by compute.) ----
        tiles = []
        for ci in range(n_chunks):
            h0 = ci * CHUNK_H
            hs = slice(h0, h0 + CHUNK_H)
            x2 = p2.tile([128, F], f32)
            nc.sync.dma_start(
                out=x2[:],
                in_=x23[:, 0:32, hs, :].rearrange("n c h w -> c n (h w)"),
            )
            x3 = p3.tile([128, F], f32)
            nc.scalar.dma_start(
                out=x3[:],
                in_=x23[:, 32:64, hs, :].rearrange("n c h w -> c n (h w)"),
            )
            x1a = p1.tile([128, F], f16, tag="g1a")
            nc.sync.dma_start(
                out=x1a[:],
                in_=x1[0:2, :, hs, :].rearrange("n c h w -> c n (h w)"),
            )
            x1b = p1.tile([128, F], f16, tag="g1b")
            nc.scalar.dma_start(
                out=x1b[:],
                in_=x1[2:4, :, hs, :].rearrange("n c h w -> c n (h w)"),
            )
            tiles.append((x2, x3, x1a, x1b))

        for ci in range(n_chunks):
            h0 = ci * CHUNK_H
            hs = slice(h0, h0 + CHUNK_H)
            x2, x3, x1a, x1b = tiles[ci]

            # ---- G1 relu on ACT (fp16 in-place), store via SWDGE ----
            for xt, ns in ((x1a, slice(0, 2)), (x1b, slice(2, 4))):
                nc.scalar.activation(
                    xt[:], xt[:], mybir.ActivationFunctionType.Relu
                )
                nc.gpsimd.dma_start(
                    out=y[ns, 0:64, hs, :].rearrange("n c h w -> c n (h w)"),
                    in_=xt[:],
                )

            # ---- G2: 2x2 blocks, channels [64,96) ----
            # Mask apply uses a stride-0 broadcast view of the block sums so
            # the out/in1 access is row-strided with a contiguous 112-elem
            # inner run ([w2=56, j=2] fuses); the old per-(i,j) w-strided
            # stts ran at ~50% DVE efficiency.
            x2v = x2[:].rearrange("p (h w) -> p h w", h=CHUNK_H)
            o2 = po.tile([128, F], f16, tag="o2")
            o2v = o2[:].rearrange("p (h w) -> p h w", h=CHUNK_H)
            s1 = tp.tile([128, CHUNK_H * (W // 2)], f32, tag="s1")
            s1v = s1[:].rearrange("p (h w) -> p h w", h=CHUNK_H)
            nc.vector.tensor_add(s1v, x2v[:, :, 0::2], x2v[:, :, 1::2])
            s2 = tp.tile([128, (CHUNK_H // 2) * (W // 2)], f32, tag="s2")
            s2v = s2[:].rearrange("p (h w) -> p h w", h=CHUNK_H // 2)
            nc.vector.tensor_add(s2v, s1v[:, 0::2, :], s1v[:, 1::2, :])
            s2b = s2v.unsqueeze(3).broadcast_to(
                [128, CHUNK_H // 2, W // 2, 2]
            )
            for i in range(2):
                nc.vector.scalar_tensor_tensor(
                    o2v[:, i::2, :].rearrange("p h (w j) -> p h w j", j=2),
                    s2b, 0.0,
                    x2v[:, i::2, :].rearrange("p h (w j) -> p h w j", j=2),
                    ge, mul,
                )
            nc.gpsimd.dma_start(
                out=y[:, 64:96, hs, :].rearrange("n c h w -> c n (h w)"),
                in_=o2[:],
            )

            # ---- G3: 4x4 blocks [96,120) + identity [120,128) ----
            x3v = x3[0:96].rearrange("p (h w) -> p h w", h=CHUNK_H)
            o3 = po.tile([128, F], f16, tag="o3")
            o3v = o3[0:96].rearrange("p (h w) -> p h w", h=CHUNK_H)
            t1 = tp.tile([96, CHUNK_H * (W // 2)], f32, tag="s1")
            t1v = t1[:].rearrange("p (h w) -> p h w", h=CHUNK_H)
            nc.vector.tensor_add(t1v, x3v[:, :, 0::2], x3v[:, :, 1::2])
            t2 = tp.tile([96, CHUNK_H * (W // 4)], f32, tag="s2")
            t2v = t2[:].rearrange("p (h w) -> p h w", h=CHUNK_H)
            nc.vector.tensor_add(t2v, t1v[:, :, 0::2], t1v[:, :, 1::2])
            t3 = tp.tile([96, (CHUNK_H // 2) * (W // 4)], f32, tag="t3")
            t3v = t3[:].rearrange("p (h w) -> p h w", h=CHUNK_H // 2)
            nc.vector.tensor_add(t3v, t2v[:, 0::2, :], t2v[:, 1::2, :])
            t4 = tp.tile([96, (CHUNK_H // 4) * (W // 4)], f32, tag="t4")
            t4v = t4[:].rearrange("p (h w) -> p h w", h=CHUNK_H // 4)
            nc.vector.tensor_add(t4v, t3v[:, 0::2, :], t3v[:, 1::2, :])
            t4b = t4v.unsqueeze(3).broadcast_to(
                [96, CHUNK_H // 4, W // 4, 4]
            )
            for i in range(4):
                nc.vector.scalar_tensor_tensor(
                    o3v[:, i::4, :].rearrange("p h (w j) -> p h w j", j=4),
                    t4b, 0.0,
                    x3v[:, i::4, :].rearrange("p h (w j) -> p h w j", j=4),
                    ge, mul,
                )
            # identity channels: f32 -> fp16 copy on ScalarE
            nc.scalar.activation(
                o3[96:128], x3[96:128], mybir.ActivationFunctionType.Copy
            )
            nc.gpsimd.dma_start(
                out=y[:, 96:128, hs, :].rearrange("n c h w -> c n (h w)"),
                in_=o3[:],
            )

    nc.compile()
    return nc


def _get_compiled():
    global _compiled
    if _compiled is None:
        _compiled = _build()
    return _compiled


def kernel(activation: np.ndarray, _trace: bool = False):
    nc = _get_compiled()
    activation = np.ascontiguousarray(activation, dtype=np.float32)
    in_maps = []
    for i in range(N_CORES):
        sl = activation[i * N_PER_CORE : (i + 1) * N_PER_CORE]
        in_maps.append({
            "x1": sl[:, 0:64].astype(np.float16),
            "x23": np.ascontiguousarray(sl[:, 64:128]),
        })
    res = run_bass_kernel_spmd(nc, in_maps, core_ids=list(range(N_CORES)),
                               trace=_trace)
    out = np.concatenate([r["y"] for r in res.results], axis=0)
    out = out.astype(np.float32)
    if _trace:
        return out, res
    return out


# revision 5
# speedup vs baseline: 1.3119x; 1.3119x over previous
"""BlockReLU Trainium2 kernel.

Full input: activation [32, 128, 112, 112] f32. Channel groups:
  [0,64): 1x1 blocks (plain ReLU), [64,96): 2x2 blocks, [96,120): 4x4 blocks,
  [120,128): identity passthrough.
A block's mask is 1 where the block's spatial sum >= 0, else 0; the mask is
broadcast over the block and multiplies the input.

Strategy: pure data parallelism over batch N across 8 NeuronCores (4 images
per core). The problem is HBM-bandwidth-bound (baseline f32-in/f32-out ran at
~374 GB/s/core, at the shared-HBM-stack roofline), so the win is TRAFFIC
REDUCTION under the 2e-2 rel-err budget:
  - All OUTPUT is written fp16 (rel err 2^-11 = 4.9e-4; host upcasts to f32).
  - G1 (1x1/ReLU, channels [0,64) = half the tensor) INPUT is fp16
    (host pre-casts): a 1x1 "mask flip" only happens when |x| is at fp16
    rounding scale, so the error stays ~5e-4*|x|.
  - G2/G3 inputs stay f32: their masks need exact block sums -- an fp16-
    perturbed sum near 0 would flip a whole block's mask and zero/unzero
    O(1)-magnitude elements (max-norm rel err ~0.5).
Traffic/core: 19.25 MB loads + 12.8 MB stores = 32.1 MB vs 51.4 MB baseline.

Per core, stream H in chunks of CHUNK_H rows. For each chunk, pack
(channel, image) pairs onto all 128 SBUF partitions so every engine op uses
every lane:
  G1 relu:  two fp16 tiles, p = c*2 + n_local (64ch x 2img) -> ScalarE Relu
            in-place, stored straight from the input tile.
  G2 2x2:   one f32 tile,  p = (c-64)*4 + n  (32ch x 4img) -> VectorE,
            masked product written to an fp16 out tile (conversion fused
            into the scalar_tensor_tensor write port).
  G3+G4:    one f32 tile,  p = (c-96)*4 + n  (32ch x 4img);
            partitions 0:96 are the 4x4-mask channels (VectorE), 96:128
            identity channels copied f32->fp16 on ScalarE.
Block sums are strided pairwise tensor_adds in f32; mask apply is a fused
scalar_tensor_tensor: out_fp16 = (sum >= 0) * x_f32.

Engine/queue layout (measured on HW in the f32 baseline; kept):
  - Loads split across both HWDGE rings (g2+g1a on nc.sync/SP, g3+g1b on
    nc.scalar/ACT). Stores kept OFF these rings: a store's sem-wait
    head-of-line-blocks later descriptor pushes, starving the load stream.
  - All stores on the SWDGE ring (nc.gpsimd.dma_start) - descriptor
    generation only.
  - Deep prefetch: input pools hold all 7 chunks (bufs=7) so loads never
    wait on store completions.
  - All DMAs are 128-partition; 96-partition DMAs cap at 12/16 SBUF ports
    and poison aggregate bandwidth (measured), hence the identity channels
    ride inside the G3 tile.
"""
import sys

if "/opt/trn_rl_repo" not in sys.path:
    sys.path.insert(0, "/opt/trn_rl_repo")

import numpy as np
from contextlib import ExitStack

import concourse.tile as tile
from concourse import bacc, mybir
from concourse.bass_utils import run_bass_kernel_spmd

N_FULL, C, H, W = 32, 128, 112, 112
N_CORES = 8
N_PER_CORE = N_FULL // N_CORES  # 4
CHUNK_H = 16

_compiled = None


def _build():
    N = N_PER_CORE
    f32 = mybir.dt.float32
    f16 = mybir.dt.float16
    nc = bacc.Bacc("TRN2", target_bir_lowering=False, debug=False)
    x1 = nc.dram_tensor("x1", [N, 64, H, W], f16, kind="ExternalInput").ap()
    x23 = nc.dram_tensor("x23", [N, 64, H, W], f32, kind="ExternalInput").ap()
    y = nc.dram_tensor("y", [N, C, H, W], f16, kind="ExternalOutput").ap()

    n_chunks = H // CHUNK_H
    F = CHUNK_H * W
    ge, mul = mybir.AluOpType.is_ge, mybir.AluOpType.mult

    with tile.TileContext(nc) as tc, ExitStack() as ctx:
        p1 = ctx.enter_context(tc.tile_pool(name="g1", bufs=7))
        p2 = ctx.enter_context(tc.tile_pool(name="g2", bufs=7))
        p3 = ctx.enter_context(tc.tile_pool(name="g3", bufs=7))
        po = ctx.enter_context(tc.tile_pool(name="out", bufs=7))
        tp = ctx.enter_context(tc.tile_pool(name="tmp", bufs=1))

        # Loads are pushed LOOKAHEAD chunks ahead of compute. Two failure
        # modes bracket this choice (both measured): pushing everything
        # upfront keeps both HWDGE load queues permanently full, and the
        # per-SDMA-engine packet round-robin then gives the single SWDGE
        # store queue only ~25% of slots (needs 40%) — stores starve and
        # stall compute via the out pool. Pushing per-chunk lets the ACT
        # engine's relu ACTIVATEs head-of-line-block that ring's pushes
        # (load span 51us -> 68us). Lookahead=2 keeps pushes ahead of the
        # compute that would block them while letting load queues drain so
        # stores get arbitration slots.
        def emit_loads(ci):
            h0 = ci * CHUNK_H
            hs = slice(h0, h0 + CHUNK_H)
            x2 = p2.tile([128, F], f32)
            nc.sync.dma_start(
                out=x2[:],
                in_=x23[:, 0:32, hs, :].rearrange("n c h w -> c n (h w)"),
            )
            x3 = p3.tile([128, F], f32)
            nc.scalar.dma_start(
                out=x3[:],
                in_=x23[:, 32:64, hs, :].rearrange("n c h w -> c n (h w)"),
            )
            x1a = p1.tile([128, F], f16, tag="g1a")
            nc.sync.dma_start(
                out=x1a[:],
                in_=x1[0:2, :, hs, :].rearrange("n c h w -> c n (h w)"),
            )
            x1b = p1.tile([128, F], f16, tag="g1b")
            nc.scalar.dma_start(
                out=x1b[:],
                in_=x1[2:4, :, hs, :].rearrange("n c h w -> c n (h w)"),
            )
            return (x2, x3, x1a, x1b)

        LOOKAHEAD = 2
        tiles = {ci: emit_loads(ci) for ci in range(min(LOOKAHEAD + 1, n_chunks))}

        for ci in range(n_chunks):
            if ci + LOOKAHEAD + 1 < n_chunks:
                tiles[ci + LOOKAHEAD + 1] = emit_loads(ci + LOOKAHEAD + 1)
            h0 = ci * CHUNK_H
            hs = slice(h0, h0 + CHUNK_H)
            x2, x3, x1a, x1b = tiles.pop(ci)

            # ---- G1 relu on ACT (fp16 in-place), store via SWDGE ----
            for xt, ns in ((x1a, slice(0, 2)), (x1b, slice(2, 4))):
                nc.scalar.activation(
                    xt[:], xt[:], mybir.ActivationFunctionType.Relu
                )
                nc.gpsimd.dma_start(
                    out=y[ns, 0:64, hs, :].rearrange("n c h w -> c n (h w)"),
                    in_=xt[:],
                )

            # ---- G2: 2x2 blocks, channels [64,96) ----
            # Mask apply uses a stride-0 broadcast view of the block sums so
            # the out/in1 access is row-strided with a contiguous 112-elem
            # inner run ([w2=56, j=2] fuses); the old per-(i,j) w-strided
            # stts ran at ~50% DVE efficiency.
            x2v = x2[:].rearrange("p (h w) -> p h w", h=CHUNK_H)
            o2 = po.tile([128, F], f16, tag="o2")
            o2v = o2[:].rearrange("p (h w) -> p h w", h=CHUNK_H)
            s1 = tp.tile([128, CHUNK_H * (W // 2)], f32, tag="s1")
            s1v = s1[:].rearrange("p (h w) -> p h w", h=CHUNK_H)
            nc.vector.tensor_add(s1v, x2v[:, :, 0::2], x2v[:, :, 1::2])
            s2 = tp.tile([128, (CHUNK_H // 2) * (W // 2)], f32, tag="s2")
            s2v = s2[:].rearrange("p (h w) -> p h w", h=CHUNK_H // 2)
            nc.vector.tensor_add(s2v, s1v[:, 0::2, :], s1v[:, 1::2, :])
            s2b = s2v.unsqueeze(3).broadcast_to(
                [128, CHUNK_H // 2, W // 2, 2]
            )
            for i in range(2):
                nc.vector.scalar_tensor_tensor(
                    o2v[:, i::2, :].rearrange("p h (w j) -> p h w j", j=2),
                    s2b, 0.0,
                    x2v[:, i::2, :].rearrange("p h (w j) -> p h w j", j=2),
                    ge, mul,
                )
            nc.gpsimd.dma_start(
                out=y[:, 64:96, hs, :].rearrange("n c h w -> c n (h w)"),
                in_=o2[:],
            )

            # ---- G3: 4x4 blocks [96,120) + identity [120,128) ----
            x3v = x3[0:96].rearrange("p (h w) -> p h w", h=CHUNK_H)
            o3 = po.tile([128, F], f16, tag="o3")
            o3v = o3[0:96].rearrange("p (h w) -> p h w", h=CHUNK_H)
            t1 = tp.tile([96, CHUNK_H * (W // 2)], f32, tag="s1")
            t1v = t1[:].rearrange("p (h w) -> p h w", h=CHUNK_H)
            nc.vector.tensor_add(t1v, x3v[:, :, 0::2], x3v[:, :, 1::2])
            t2 = tp.tile([96, CHUNK_H * (W // 4)], f32, tag="s2")
            t2v = t2[:].rearrange("p (h w) -> p h w", h=CHUNK_H)
            nc.vector.tensor_add(t2v, t1v[:, :, 0::2], t1v[:, :, 1::2])
            t3 = tp.tile([96, (CHUNK_H // 2) * (W // 4)], f32, tag="t3")
            t3v = t3[:].rearrange("p (h w) -> p h w", h=CHUNK_H // 2)
            nc.vector.tensor_add(t3v, t2v[:, 0::2, :], t2v[:, 1::2, :])
            t4 = tp.tile([96, (CHUNK_H // 4) * (W // 4)], f32, tag="t4")
            t4v = t4[:].rearrange("p (h w) -> p h w", h=CHUNK_H // 4)
            nc.vector.tensor_add(t4v, t3v[:, 0::2, :], t3v[:, 1::2, :])
            t4b = t4v.unsqueeze(3).broadcast_to(
                [96, CHUNK_H // 4, W // 4, 4]
            )
            for i in range(4):
                nc.vector.scalar_tensor_tensor(
                    o3v[:, i::4, :].rearrange("p h (w j) -> p h w j", j=4),
                    t4b, 0.0,
                    x3v[:, i::4, :].rearrange("p h (w j) -> p h w j", j=4),
                    ge, mul,
                )
            # identity channels: f32 -> fp16 copy on ScalarE
            nc.scalar.activation(
                o3[96:128], x3[96:128], mybir.ActivationFunctionType.Copy
            )
            nc.gpsimd.dma_start(
                out=y[:, 96:128, hs, :].rearrange("n c h w -> c n (h w)"),
                in_=o3[:],
            )

    nc.compile()
    return nc


def _get_compiled():
    global _compiled
    if _compiled is None:
        _compiled = _build()
    return _compiled


def kernel(activation: np.ndarray, _trace: bool = False):
    nc = _get_compiled()
    activation = np.ascontiguousarray(activation, dtype=np.float32)
    in_maps = []
    for i in range(N_CORES):
        sl = activation[i * N_PER_CORE : (i + 1) * N_PER_CORE]
        in_maps.append({
            "x1": sl[:, 0:64].astype(np.float16),
            "x23": np.ascontiguousarray(sl[:, 64:128]),
        })
    res = run_bass_kernel_spmd(nc, in_maps, core_ids=list(range(N_CORES)),
                               trace=_trace)
    out = np.concatenate([r["y"] for r in res.results], axis=0)
    out = out.astype(np.float32)
    if _trace:
        return out, res
    return out
